# revision 20
# baseline (speedup 1.0000x reference)
"""Trainium2 Bass kernel: pre-LN multi-head attention (B=2, S=2048, d_model=1024, H=16).

Sharding: 8 cores = 2 batches x 4 head-groups. Core c handles batch c//4 and
heads 4*(c%4) .. 4*(c%4)+3 (a 256-wide slice of d_model).

Per-core device pipeline (all shapes per core):
  x_{q,k,v} [2048,1024]  --LN stats (DVE) + normalize (GPSIMD)-->  z  (token-major)
  z --PE transpose--> zT [1024,2048] (feature-major)
  QT/KT [256,2048] = W_slice @ zT   (feature-major, PE)
  V     [2048,256] token-major (zT as stationary), stored interleaved with a
        ones-column per head so the AV matmul also produces softmax denominators
  S^T   [k,q] tiles = K_h @ Q_h^T ; P^T = exp(S^T/8) (ACT, no max-subtraction:
        scores are ~N(0,1), exp is safe in fp32)
  ctx^T [64,q] = V_h^T @ P^T (fp32 PSUM accum; row 64 = sum_k P = denominator)
  y_partial [2048,1024] = ctx^T.T @ fo_slice^T  (PSUM -> DRAM)

Host: LayerNorm gamma/beta are folded into the projection weights/biases,
weights are pre-transposed to [in,out]; the 4 partial outputs per batch are
summed (row-parallel matmul gather-reduce) and fo_b added.
"""

import os
import numpy as np
import ml_dtypes
from contextlib import ExitStack

import concourse.bass as bass
import concourse.bacc as bacc
import concourse.tile as tile
from concourse import mybir
from concourse import bass_utils
from concourse.masks import make_identity

F32 = mybir.dt.float32
BF16 = mybir.dt.bfloat16

# All transcendentals in this kernel are Exp/Ln (rstd = exp(-0.5*ln(var+eps)),
# softmax exp, 1/denom = exp(-ln(d))). Exp and Ln coexist in the
# `natural_log_exp_and_others` ACT table set, but the table chooser picks
# per-function sets, emitting an ~2.7us ACT_TABLE_LOAD on every Exp<->Ln
# alternation. Strip Exp/Ln from every other set (names/indices preserved)
# so one resident set serves the whole kernel.
_orig_get_tables = bacc.get_activation_tables
_COMBINED = "natural_log_exp_and_others"


def _patched_get_tables(arch):
    tabs = _orig_get_tables(arch)
    if _COMBINED in tabs:
        drop = {mybir.ActivationFunctionType.Exp, mybir.ActivationFunctionType.Ln}
        tabs = {
            name: (fns if name == _COMBINED else fns - drop)
            for name, fns in tabs.items()
        }
    return tabs


bacc.get_activation_tables = _patched_get_tables

# ---- problem constants (hardcoded; kernel.py must be self-contained) ----
B, S, D = 2, 2048, 1024
NH_TOT, DH = 16, 64
N_CORES = 8
HPC = NH_TOT // 4          # 4 heads per core
HS = HPC * DH              # 256-wide feature slice per core
NT = S // 128              # 16 token tiles
NIC = D // 128             # 8 input-feature chunks
QB = 512                   # q-block width for attention
NQB = S // QB              # 4
LN_EPS = 1e-5
SCALE = 1.0 / np.sqrt(DH)  # 0.125

# dtype config (env-overridable for experiments)
_DT = {"f32": F32, "bf16": BF16}
XDT = _DT[os.environ.get("K_XDT", "bf16")]   # x input dtype (LN stats input)
TDT = _DT[os.environ.get("K_TDT", "bf16")]   # z / zT dtype (projection inputs)
WDT = _DT[os.environ.get("K_WDT", "bf16")]   # weight dtype
MDT = _DT[os.environ.get("K_MDT", "bf16")]   # attention matmul input dtype (QT/KT/V/P/CT)
TRANS_ENG = os.environ.get("K_TRANS", "dma")  # 'dma' (xbar) or 'pe' transposes

_NPDT = {F32: np.float32, BF16: ml_dtypes.bfloat16}


def build_nc():
    nc = bacc.Bacc("TRN2", target_bir_lowering=False, debug=False)

    xq = nc.dram_tensor("xq", [S, D], XDT, kind="ExternalInput")
    xk = nc.dram_tensor("xk", [S, D], XDT, kind="ExternalInput")
    xv = nc.dram_tensor("xv", [S, D], XDT, kind="ExternalInput")
    wq = nc.dram_tensor("wq", [D, HS], WDT, kind="ExternalInput")
    wk = nc.dram_tensor("wk", [D, HS], WDT, kind="ExternalInput")
    wv = nc.dram_tensor("wv", [D, HS], WDT, kind="ExternalInput")
    fo = nc.dram_tensor("fo", [HS, D], WDT, kind="ExternalInput")
    bq = nc.dram_tensor("bq", [128, HS // 128], F32, kind="ExternalInput")
    bk = nc.dram_tensor("bk", [128, HS // 128], F32, kind="ExternalInput")
    bv = nc.dram_tensor("bv", [1, HS], F32, kind="ExternalInput")
    y = nc.dram_tensor("y", [S, D], F32, kind="ExternalOutput")

    with tile.TileContext(nc) as tc, ExitStack() as ctx:
        singles = ctx.enter_context(tc.tile_pool(name="singles", bufs=1))
        xp = ctx.enter_context(tc.tile_pool(name="xp", bufs=6))
        zp = ctx.enter_context(tc.tile_pool(name="zp", bufs=8))
        statp = ctx.enter_context(tc.tile_pool(name="statp", bufs=8))
        ztp = ctx.enter_context(tc.tile_pool(name="ztp", bufs=2))
        pp_mm = ctx.enter_context(
            tc.tile_pool(name="pp_mm", bufs=2, space="PSUM"))
        pp_st = ctx.enter_context(
            tc.tile_pool(name="pp_st", bufs=2, space="PSUM"))
        pp_av = ctx.enter_context(
            tc.tile_pool(name="pp_av", bufs=2, space="PSUM"))
        pb = ctx.enter_context(tc.tile_pool(name="pb", bufs=10))
        recp = ctx.enter_context(tc.tile_pool(name="recp", bufs=2))
        yp = ctx.enter_context(tc.tile_pool(name="yp", bufs=2))

        # --- constants ---
        identity = singles.tile([128, 128], TDT)
        make_identity(nc, identity[:, :])
        eps_t = singles.tile([128, 1], F32)
        nc.vector.memset(eps_t[:, :], LN_EPS)
        bq_sb = singles.tile([128, 2], F32)
        nc.sync.dma_start(out=bq_sb[:, :], in_=bq[:, :])
        bk_sb = singles.tile([128, 2], F32)
        nc.sync.dma_start(out=bk_sb[:, :], in_=bk[:, :])
        bv_sb = singles.tile([128, HS], F32)
        nc.sync.dma_start(out=bv_sb[:, :], in_=bv[:, :].to_broadcast([128, HS]))

        w_sbs = {}
        for name, wd in (("q", wq), ("k", wk), ("v", wv)):
            w_sb = singles.tile([128, NIC, HS], WDT, tag=f"w{name}")
            nc.sync.dma_start(
                out=w_sb[:, :, :],
                in_=wd[:, :].rearrange("(c p) n -> p c n", p=128))
            w_sbs[name] = w_sb
        fo_sb = singles.tile([128, 2, D], WDT)
        nc.sync.dma_start(
            out=fo_sb[:, :, :], in_=fo[:, :].rearrange("(c p) n -> p c n", p=128))

        # feature-major Q^T / K^T [256, 2048] as [128, chunk, tok]
        QT = singles.tile([128, 2, S], MDT, tag="QT")
        KT = singles.tile([128, 2, S], MDT, tag="KT")
        # token-major V, heads interleaved with a ones column: [128, tok_tile, h, 65]
        V65 = singles.tile([128, NT, HPC, DH + 1], MDT, tag="V65")
        nc.vector.memset(V65[:, :, :, DH:DH + 1], 1.0)
        # feature-major context [256, 2048]
        CT = singles.tile([128, 2, S], MDT, tag="CT")

        def ln_transpose_project(x_dram, wname, mode, copy_eng):
            """Returns do_group(g): LN -> z -> zT -> projection for token tiles
            4g..4g+3. mode: 'fm' (feature-major out into QT/KT) or 'tm'
            (token-major out into V65). copy_eng: 'act' or 'dve' for the
            PSUM->SBUF transpose copyback."""
            zt = ztp.tile([128, NIC, S], TDT, tag="zt")
            w_sb = w_sbs[wname]

            def proj_group(n):
                # token-range n*512:(n+1)*512 of zT is complete
                if mode == "fm":
                    dst = QT if wname == "q" else KT
                    b_sb = bq_sb if wname == "q" else bk_sb
                    for m in range(2):
                        ps = pp_mm.tile([128, 512], F32, tag="mm")
                        for ic in range(NIC):
                            nc.tensor.matmul(
                                ps[:, :],
                                lhsT=w_sb[:, ic, m * 128:(m + 1) * 128],
                                rhs=zt[:, ic, n * 512:(n + 1) * 512],
                                start=(ic == 0), stop=(ic == NIC - 1))
                        nc.vector.tensor_scalar(
                            out=dst[:, m, n * 512:(n + 1) * 512], in0=ps[:, :],
                            scalar1=b_sb[:, m:m + 1],
                            scalar2=None, op0=mybir.AluOpType.add)
                else:
                    for j in range(4 * n, 4 * n + 4):
                        ps = pp_mm.tile([128, HS], F32, tag="mm")
                        for ic in range(NIC):
                            nc.tensor.matmul(
                                ps[:, :],
                                lhsT=zt[:, ic, j * 128:(j + 1) * 128],
                                rhs=w_sb[:, ic, :],
                                start=(ic == 0), stop=(ic == NIC - 1))
                        for h in range(HPC):
                            nc.vector.tensor_tensor(
                                out=V65[:, j, h, 0:DH],
                                in0=ps[:, h * DH:(h + 1) * DH],
                                in1=bv_sb[:, h * DH:(h + 1) * DH],
                                op=mybir.AluOpType.add)

            def do_group(grp):
                for j in range(4 * grp, 4 * grp + 4):
                    x_t = xp.tile([128, D], XDT)
                    nc.sync.dma_start(out=x_t[:, :],
                                      in_=x_dram[j * 128:(j + 1) * 128, :])
                    st = statp.tile([128, 2, 6], F32, tag="st")
                    for g in range(2):
                        nc.vector.bn_stats(out=st[:, g, :],
                                           in_=x_t[:, g * 512:(g + 1) * 512])
                    mv = statp.tile([128, 2], F32, tag="mv")
                    nc.vector.bn_aggr(out=mv[:, :], in_=st[:, :, :])
                    # rstd = exp(-0.5*ln(var+eps)) — Ln+Exp share one table set
                    lnv = statp.tile([128, 1], F32, tag="lnv")
                    nc.scalar.activation(lnv[:, :], mv[:, 1:2],
                                         mybir.ActivationFunctionType.Ln,
                                         bias=eps_t[:, :], scale=1.0)
                    rstd = statp.tile([128, 1], F32, tag="rstd")
                    nc.scalar.activation(rstd[:, :], lnv[:, :],
                                         mybir.ActivationFunctionType.Exp,
                                         scale=-0.5)
                    nmur = statp.tile([128, 1], F32, tag="nmur")
                    nc.vector.tensor_scalar(
                        out=nmur[:, :], in0=mv[:, 0:1], scalar1=rstd[:, :],
                        scalar2=-1.0, op0=mybir.AluOpType.mult,
                        op1=mybir.AluOpType.mult)
                    z = zp.tile([128, D], TDT)
                    nc.gpsimd.tensor_scalar(
                        out=z[:, :], in0=x_t[:, :], scalar1=rstd[:, :],
                        scalar2=nmur[:, :], op0=mybir.AluOpType.mult,
                        op1=mybir.AluOpType.add)
                    if TRANS_ENG == "dma":
                        # xbar transpose engine: SBUF->SBUF, no PE/ACT/DVE cost
                        for ic in range(NIC):
                            nc.sync.dma_start_transpose(
                                out=zt[:, ic, j * 128:(j + 1) * 128],
                                in_=z[:, ic * 128:(ic + 1) * 128])
                    else:
                        tp = pp_mm.tile([128, NIC, 128], TDT, tag="mm")
                        for ic in range(NIC):
                            nc.tensor.transpose(tp[:, ic, :],
                                                z[:, ic * 128:(ic + 1) * 128],
                                                identity[:, :])
                        if copy_eng == "act":
                            # Copy is present in every ACT table set (no reload)
                            nc.scalar.activation(zt[:, :, j * 128:(j + 1) * 128],
                                                 tp[:, :, :],
                                                 mybir.ActivationFunctionType.Copy)
                        else:
                            nc.vector.tensor_copy(zt[:, :, j * 128:(j + 1) * 128],
                                                  tp[:, :, :])
                proj_group(grp)

            return do_group

        # k first (full K needed by every S^T tile), then q and v interleaved
        # per 4-tile group: the attention exp stream starts as soon as QT's
        # first quarter exists, and AV(kt) streams behind V65[kt] production.
        sk = ln_transpose_project(xk, "k", "fm", "act")
        for g in range(4):
            sk(g)
        sq = ln_transpose_project(xq, "q", "fm", "dve")
        sv = ln_transpose_project(xv, "v", "tm", "dve")
        for g in range(4):
            sq(g)
            sv(g)

        # --- attention (qb outer so the output projection can stream) ---
        for qb in range(NQB):
            for h in range(HPC):
                hc, ho = h // 2, 64 * (h % 2)
                p_tiles = []
                for kt2 in range(NT // 2):
                    # two k-tiles share one 2-bank PSUM tile so a single exp
                    # covers both (amortizes the ~172-cycle ACT PSUM overhead)
                    st_ps = pp_st.tile([128, 2, QB], F32)
                    for i in range(2):
                        kt = kt2 * 2 + i
                        nc.tensor.matmul(
                            st_ps[:, i, :],
                            lhsT=KT[ho:ho + DH, hc, kt * 128:(kt + 1) * 128],
                            rhs=QT[ho:ho + DH, hc, qb * QB:(qb + 1) * QB],
                            start=True, stop=True)
                    p = pb.tile([128, 2, QB], MDT)
                    nc.scalar.activation(p[:, :, :], st_ps[:, :, :],
                                         mybir.ActivationFunctionType.Exp,
                                         scale=float(SCALE))
                    p_tiles.append(p)
                av = pp_av.tile([DH + 1, QB], F32)
                for kt in range(NT):
                    nc.tensor.matmul(
                        av[:, :],
                        lhsT=V65[:, kt, h, :],
                        rhs=p_tiles[kt // 2][:, kt % 2, :],
                        start=(kt == 0), stop=(kt == NT - 1))
                # 1/denom = exp(-ln(denom)) on ACT: avoids the 1-lane DVE
                # iterative divide (~3.3us per row) and stays in the one
                # resident Exp/Ln table set.
                lnd = recp.tile([1, QB], F32, tag="lnd")
                nc.scalar.activation(lnd[:, :], av[DH:DH + 1, :],
                                     mybir.ActivationFunctionType.Ln)
                rec = recp.tile([1, QB], F32, tag="rec")
                nc.scalar.activation(rec[:, :], lnd[:, :],
                                     mybir.ActivationFunctionType.Exp,
                                     scale=-1.0)
                recb = recp.tile([DH, QB], F32, tag="recb")
                nc.gpsimd.partition_broadcast(recb[:, :], rec[:, :])
                nc.vector.tensor_tensor(
                    out=CT[ho:ho + DH, hc, qb * QB:(qb + 1) * QB],
                    in0=av[0:DH, :], in1=recb[:, :],
                    op=mybir.AluOpType.mult)

            # output projection for this qb's token tiles (all heads done)
            for j in range(4 * qb, 4 * qb + 4):
                ys = yp.tile([128, D], F32)
                for n in range(2):
                    ps = pp_mm.tile([128, 512], F32, tag="mm")
                    for cc in range(2):
                        nc.tensor.matmul(
                            ps[:, :],
                            lhsT=CT[:, cc, j * 128:(j + 1) * 128],
                            rhs=fo_sb[:, cc, n * 512:(n + 1) * 512],
                            start=(cc == 0), stop=(cc == 1))
                    nc.vector.tensor_copy(ys[:, n * 512:(n + 1) * 512], ps[:, :])
                nc.sync.dma_start(out=y[j * 128:(j + 1) * 128, :], in_=ys[:, :])

    nc.compile()
    return nc


_NC_CACHE = {}


def _get_nc():
    key = (XDT, TDT, WDT, MDT)
    if key not in _NC_CACHE:
        _NC_CACHE[key] = build_nc()
    return _NC_CACHE[key]


def make_in_maps(q, k, v, ln_g, ln_b, wq_w, wq_b, wk_w, wk_b, wv_w, wv_b, fo_w, fo_b):
    """Host-side shard prep. Folds ln_g/ln_b into projection weights/biases."""
    xnp = _NPDT[XDT]
    wnp = _NPDT[WDT]
    g64 = ln_g.astype(np.float64)
    b64 = ln_b.astype(np.float64)
    in_maps = []
    for c in range(N_CORES):
        b = c // 4
        sl = slice((c % 4) * HS, (c % 4 + 1) * HS)
        m = {
            "xq": np.ascontiguousarray(q[b]).astype(xnp),
            "xk": np.ascontiguousarray(k[b]).astype(xnp),
            "xv": np.ascontiguousarray(v[b]).astype(xnp),
        }
        for nm, w, bias in (("q", wq_w, wq_b), ("k", wk_w, wk_b), ("v", wv_w, wv_b)):
            ws = w[sl].astype(np.float64)          # [256, 1024]
            wg = ws * g64[None, :]                 # fold gamma
            cb = (ws @ b64 + bias[sl].astype(np.float64)).astype(np.float32)
            m["w" + nm] = np.ascontiguousarray(wg.T).astype(wnp)  # [1024, 256]
            if nm == "v":
                m["bv"] = cb.reshape(1, HS)
            else:
                m["b" + nm] = np.ascontiguousarray(cb.reshape(2, 128).T)  # [128, 2]
        m["fo"] = np.ascontiguousarray(fo_w[:, sl].T).astype(wnp)  # [256, 1024]
        in_maps.append(m)
    return in_maps


def run_on_device(in_maps, trace=False):
    nc = _get_nc()
    return bass_utils.run_bass_kernel_spmd(
        nc, in_maps, core_ids=list(range(N_CORES)), trace=trace)


def assemble(res, fo_b):
    """Gather-reduce the row-parallel partials and add the output bias."""
    fo_b64 = np.asarray(fo_b, np.float64)
    out = np.empty((B, S, D), np.float32)
    for b in range(B):
        acc = np.zeros((S, D), np.float64)
        for c in range(b * 4, b * 4 + 4):
            acc += res.results[c]["y"].astype(np.float64)
        out[b] = (acc + fo_b64[None, :]).astype(np.float32)
    return out


def kernel(q, k, v, ln_g, ln_b, wq_w, wq_b, wk_w, wk_b, wv_w, wv_b, fo_w, fo_b):
    q = np.asarray(q, np.float32)
    k = np.asarray(k, np.float32)
    v = np.asarray(v, np.float32)
    in_maps = make_in_maps(q, k, v, np.asarray(ln_g, np.float32),
                           np.asarray(ln_b, np.float32),
                           np.asarray(wq_w, np.float32), np.asarray(wq_b, np.float32),
                           np.asarray(wk_w, np.float32), np.asarray(wk_b, np.float32),
                           np.asarray(wv_w, np.float32), np.asarray(wv_b, np.float32),
                           np.asarray(fo_w, np.float32), np.asarray(fo_b, np.float32))
    res = run_on_device(in_maps)
    return assemble(res, fo_b)


# revision 24
# speedup vs baseline: 2.5903x; 2.5903x over previous
"""Trainium2 Bass kernel: pre-LN multi-head attention (B=2, S=2048, d_model=1024, H=16).

Sharding: 8 cores = 2 batches x 4 head-groups. Core c handles batch c//4 and
heads 4*(c%4) .. 4*(c%4)+3 (a 256-wide slice of d_model).

Per-core device pipeline (all shapes per core):
  x_{q,k,v} [2048,1024]  --LN stats (DVE) + normalize (GPSIMD)-->  z  (token-major)
  z --PE transpose--> zT [1024,2048] (feature-major)
  QT/KT [256,2048] = W_slice @ zT   (feature-major, PE)
  V     [2048,256] token-major (zT as stationary), stored interleaved with a
        ones-column per head so the AV matmul also produces softmax denominators
  S^T   [k,q] tiles = K_h @ Q_h^T ; P^T = exp(S^T/8) (ACT, no max-subtraction:
        scores are ~N(0,1), exp is safe in fp32)
  ctx^T [64,q] = V_h^T @ P^T (fp32 PSUM accum; row 64 = sum_k P = denominator)
  y_partial [2048,1024] = ctx^T.T @ fo_slice^T  (PSUM -> DRAM)

Host: LayerNorm gamma/beta are folded into the projection weights/biases,
weights are pre-transposed to [in,out]; the 4 partial outputs per batch are
summed (row-parallel matmul gather-reduce) and fo_b added.
"""

import os
import numpy as np
import ml_dtypes
from contextlib import ExitStack

import concourse.bass as bass
import concourse.bacc as bacc
import concourse.tile as tile
from concourse import mybir
from concourse import bass_utils
from concourse.masks import make_identity

F32 = mybir.dt.float32
BF16 = mybir.dt.bfloat16

# All transcendentals in this kernel are Exp/Ln (rstd = exp(-0.5*ln(var+eps)),
# softmax exp, 1/denom = exp(-ln(d))). Exp and Ln coexist in the
# `natural_log_exp_and_others` ACT table set, but the table chooser picks
# per-function sets, emitting an ~2.7us ACT_TABLE_LOAD on every Exp<->Ln
# alternation. Strip Exp/Ln from every other set (names/indices preserved)
# so one resident set serves the whole kernel.
_orig_get_tables = bacc.get_activation_tables
_COMBINED = "natural_log_exp_and_others"


def _patched_get_tables(arch):
    tabs = _orig_get_tables(arch)
    if _COMBINED in tabs:
        drop = {mybir.ActivationFunctionType.Exp, mybir.ActivationFunctionType.Ln}
        tabs = {
            name: (fns if name == _COMBINED else fns - drop)
            for name, fns in tabs.items()
        }
    return tabs


bacc.get_activation_tables = _patched_get_tables

# ---- problem constants (hardcoded; kernel.py must be self-contained) ----
B, S, D = 2, 2048, 1024
NH_TOT, DH = 16, 64
N_CORES = 8
HPC = NH_TOT // 4          # 4 heads per core
HS = HPC * DH              # 256-wide feature slice per core
NT = S // 128              # 16 token tiles
NIC = D // 128             # 8 input-feature chunks
QB = 512                   # q-block width for attention
NQB = S // QB              # 4
LN_EPS = 1e-5
SCALE = 1.0 / np.sqrt(DH)  # 0.125

# dtype config (env-overridable for experiments)
_DT = {"f32": F32, "bf16": BF16}
XDT = _DT[os.environ.get("K_XDT", "bf16")]   # x input dtype (LN stats input)
TDT = _DT[os.environ.get("K_TDT", "bf16")]   # z / zT dtype (projection inputs)
WDT = _DT[os.environ.get("K_WDT", "bf16")]   # weight dtype
MDT = _DT[os.environ.get("K_MDT", "bf16")]   # attention matmul input dtype (QT/KT/V/P/CT)
TRANS_ENG = os.environ.get("K_TRANS", "pe")  # 'pe' or 'dma' (xbar) transposes

_NPDT = {F32: np.float32, BF16: ml_dtypes.bfloat16}


def build_nc():
    nc = bacc.Bacc("TRN2", target_bir_lowering=False, debug=False)

    xq = nc.dram_tensor("xq", [S, D], XDT, kind="ExternalInput")
    xk = nc.dram_tensor("xk", [S, D], XDT, kind="ExternalInput")
    xv = nc.dram_tensor("xv", [S, D], XDT, kind="ExternalInput")
    wq = nc.dram_tensor("wq", [D, HS], WDT, kind="ExternalInput")
    wk = nc.dram_tensor("wk", [D, HS], WDT, kind="ExternalInput")
    wv = nc.dram_tensor("wv", [D, HS], WDT, kind="ExternalInput")
    fo = nc.dram_tensor("fo", [HS, D], WDT, kind="ExternalInput")
    bq = nc.dram_tensor("bq", [128, HS // 128], F32, kind="ExternalInput")
    bk = nc.dram_tensor("bk", [128, HS // 128], F32, kind="ExternalInput")
    bv = nc.dram_tensor("bv", [1, HS], F32, kind="ExternalInput")
    y = nc.dram_tensor("y", [S, D], F32, kind="ExternalOutput")

    with tile.TileContext(nc) as tc, ExitStack() as ctx:
        singles = ctx.enter_context(tc.tile_pool(name="singles", bufs=1))
        xp = ctx.enter_context(tc.tile_pool(name="xp", bufs=9))
        zp = ctx.enter_context(tc.tile_pool(name="zp", bufs=8))
        statp = ctx.enter_context(tc.tile_pool(name="statp", bufs=8))
        ztp = ctx.enter_context(tc.tile_pool(name="ztp", bufs=2))
        pp_mm = ctx.enter_context(
            tc.tile_pool(name="pp_mm", bufs=2, space="PSUM"))
        pp_st = ctx.enter_context(
            tc.tile_pool(name="pp_st", bufs=2, space="PSUM"))
        pp_av = ctx.enter_context(
            tc.tile_pool(name="pp_av", bufs=2, space="PSUM"))
        pb = ctx.enter_context(tc.tile_pool(name="pb", bufs=10))
        recp = ctx.enter_context(tc.tile_pool(name="recp", bufs=2))
        yp = ctx.enter_context(tc.tile_pool(name="yp", bufs=2))

        # --- constants ---
        identity = singles.tile([128, 128], TDT)
        make_identity(nc, identity[:, :])
        eps_t = singles.tile([128, 1], F32)
        nc.vector.memset(eps_t[:, :], LN_EPS)
        bq_sb = singles.tile([128, 2], F32)
        nc.sync.dma_start(out=bq_sb[:, :], in_=bq[:, :])
        bk_sb = singles.tile([128, 2], F32)
        nc.sync.dma_start(out=bk_sb[:, :], in_=bk[:, :])
        bv_sb = singles.tile([128, HS], F32)
        nc.sync.dma_start(out=bv_sb[:, :], in_=bv[:, :].to_broadcast([128, HS]))

        w_sbs = {}
        for name, wd in (("q", wq), ("k", wk), ("v", wv)):
            w_sb = singles.tile([128, NIC, HS], WDT, tag=f"w{name}")
            nc.sync.dma_start(
                out=w_sb[:, :, :],
                in_=wd[:, :].rearrange("(c p) n -> p c n", p=128))
            w_sbs[name] = w_sb
        fo_sb = singles.tile([128, 2, D], WDT)
        nc.sync.dma_start(
            out=fo_sb[:, :, :], in_=fo[:, :].rearrange("(c p) n -> p c n", p=128))

        # feature-major Q^T / K^T [256, 2048] as [128, chunk, tok]
        QT = singles.tile([128, 2, S], MDT, tag="QT")
        KT = singles.tile([128, 2, S], MDT, tag="KT")
        # token-major V, heads interleaved with a ones column: [128, tok_tile, h, 65]
        V65 = singles.tile([128, NT, HPC, DH + 1], MDT, tag="V65")
        nc.vector.memset(V65[:, :, :, DH:DH + 1], 1.0)
        # feature-major context [256, 2048]
        CT = singles.tile([128, 2, S], MDT, tag="CT")

        def ln_transpose_project(x_dram, wname, mode, copy_eng):
            """Returns do_group(g): LN -> z -> zT -> projection for token tiles
            4g..4g+3. mode: 'fm' (feature-major out into QT/KT) or 'tm'
            (token-major out into V65). copy_eng: 'act' or 'dve' for the
            PSUM->SBUF transpose copyback."""
            zt = ztp.tile([128, NIC, S], TDT, tag="zt")
            w_sb = w_sbs[wname]

            def proj_group(n):
                # token-range n*512:(n+1)*512 of zT is complete
                if mode == "fm":
                    dst = QT if wname == "q" else KT
                    b_sb = bq_sb if wname == "q" else bk_sb
                    for m in range(2):
                        ps = pp_mm.tile([128, 512], F32, tag="mm")
                        for ic in range(NIC):
                            nc.tensor.matmul(
                                ps[:, :],
                                lhsT=w_sb[:, ic, m * 128:(m + 1) * 128],
                                rhs=zt[:, ic, n * 512:(n + 1) * 512],
                                start=(ic == 0), stop=(ic == NIC - 1))
                        nc.vector.tensor_scalar(
                            out=dst[:, m, n * 512:(n + 1) * 512], in0=ps[:, :],
                            scalar1=b_sb[:, m:m + 1],
                            scalar2=None, op0=mybir.AluOpType.add)
                else:
                    for j in range(4 * n, 4 * n + 4):
                        ps = pp_mm.tile([128, HS], F32, tag="mm")
                        for ic in range(NIC):
                            nc.tensor.matmul(
                                ps[:, :],
                                lhsT=zt[:, ic, j * 128:(j + 1) * 128],
                                rhs=w_sb[:, ic, :],
                                start=(ic == 0), stop=(ic == NIC - 1))
                        for h in range(HPC):
                            nc.vector.tensor_tensor(
                                out=V65[:, j, h, 0:DH],
                                in0=ps[:, h * DH:(h + 1) * DH],
                                in1=bv_sb[:, h * DH:(h + 1) * DH],
                                op=mybir.AluOpType.add)

            def do_group(grp):
                # stats for the 4 tiles first (one batched rstd ln/exp pair
                # per group instead of per tile), then normalize+transpose
                xts = []
                mvg = statp.tile([128, 4, 2], F32, tag="mv")
                for jj, j in enumerate(range(4 * grp, 4 * grp + 4)):
                    x_t = xp.tile([128, D], XDT)
                    nc.sync.dma_start(out=x_t[:, :],
                                      in_=x_dram[j * 128:(j + 1) * 128, :])
                    xts.append(x_t)
                    st = statp.tile([128, 2, 6], F32, tag="st")
                    for g in range(2):
                        nc.vector.bn_stats(out=st[:, g, :],
                                           in_=x_t[:, g * 512:(g + 1) * 512])
                    nc.vector.bn_aggr(out=mvg[:, jj, :], in_=st[:, :, :])
                # rstd = exp(-0.5*ln(var+eps)) — Ln+Exp share one table set
                lnv = statp.tile([128, 4], F32, tag="lnv")
                nc.scalar.activation(lnv[:, :], mvg[:, :, 1],
                                     mybir.ActivationFunctionType.Ln,
                                     bias=eps_t[:, :], scale=1.0)
                rstd = statp.tile([128, 4], F32, tag="rstd")
                nc.scalar.activation(rstd[:, :], lnv[:, :],
                                     mybir.ActivationFunctionType.Exp,
                                     scale=-0.5)
                nmur = statp.tile([128, 4], F32, tag="nmur")
                nc.vector.tensor_tensor(
                    out=nmur[:, :], in0=mvg[:, :, 0], in1=rstd[:, :],
                    op=mybir.AluOpType.mult)
                nc.vector.tensor_scalar_mul(out=nmur[:, :], in0=nmur[:, :],
                                            scalar1=-1.0)
                for jj, j in enumerate(range(4 * grp, 4 * grp + 4)):
                    x_t = xts[jj]
                    z = zp.tile([128, D], TDT)
                    nc.gpsimd.tensor_scalar(
                        out=z[:, :], in0=x_t[:, :], scalar1=rstd[:, jj:jj + 1],
                        scalar2=nmur[:, jj:jj + 1], op0=mybir.AluOpType.mult,
                        op1=mybir.AluOpType.add)
                    if TRANS_ENG == "dma":
                        # xbar transpose engine: SBUF->SBUF, no PE/ACT/DVE cost
                        for ic in range(NIC):
                            nc.sync.dma_start_transpose(
                                out=zt[:, ic, j * 128:(j + 1) * 128],
                                in_=z[:, ic * 128:(ic + 1) * 128])
                    else:
                        tp = pp_mm.tile([128, NIC, 128], TDT, tag="mm")
                        for ic in range(NIC):
                            nc.tensor.transpose(tp[:, ic, :],
                                                z[:, ic * 128:(ic + 1) * 128],
                                                identity[:, :])
                        if copy_eng == "act":
                            # Copy is present in every ACT table set (no reload)
                            nc.scalar.activation(zt[:, :, j * 128:(j + 1) * 128],
                                                 tp[:, :, :],
                                                 mybir.ActivationFunctionType.Copy)
                        else:
                            nc.vector.tensor_copy(zt[:, :, j * 128:(j + 1) * 128],
                                                  tp[:, :, :])
                proj_group(grp)

            return do_group

        # k first (full K needed by every S^T tile), then q and v interleaved
        # per 4-tile group: the attention exp stream starts as soon as QT's
        # first quarter exists, and AV(kt) streams behind V65[kt] production.
        sk = ln_transpose_project(xk, "k", "fm", "act")
        for g in range(4):
            sk(g)
        sq = ln_transpose_project(xq, "q", "fm", "dve")
        sv = ln_transpose_project(xv, "v", "tm", "dve")
        for g in range(4):
            sq(g)
            sv(g)

        # --- attention (qb outer so the output projection can stream) ---
        for qb in range(NQB):
            for h in range(HPC):
                hc, ho = h // 2, 64 * (h % 2)
                p_tiles = []
                for kt2 in range(NT // 2):
                    # two k-tiles share one 2-bank PSUM tile so a single exp
                    # covers both (amortizes the ~172-cycle ACT PSUM overhead)
                    st_ps = pp_st.tile([128, 2, QB], F32)
                    for i in range(2):
                        kt = kt2 * 2 + i
                        nc.tensor.matmul(
                            st_ps[:, i, :],
                            lhsT=KT[ho:ho + DH, hc, kt * 128:(kt + 1) * 128],
                            rhs=QT[ho:ho + DH, hc, qb * QB:(qb + 1) * QB],
                            start=True, stop=True)
                    p = pb.tile([128, 2, QB], MDT)
                    nc.scalar.activation(p[:, :, :], st_ps[:, :, :],
                                         mybir.ActivationFunctionType.Exp,
                                         scale=float(SCALE))
                    p_tiles.append(p)
                av = pp_av.tile([DH + 1, QB], F32)
                for kt in range(NT):
                    nc.tensor.matmul(
                        av[:, :],
                        lhsT=V65[:, kt, h, :],
                        rhs=p_tiles[kt // 2][:, kt % 2, :],
                        start=(kt == 0), stop=(kt == NT - 1))
                # 1/denom = exp(-ln(denom)) on ACT: avoids the 1-lane DVE
                # iterative divide (~3.3us per row) and stays in the one
                # resident Exp/Ln table set.
                lnd = recp.tile([1, QB], F32, tag="lnd")
                nc.scalar.activation(lnd[:, :], av[DH:DH + 1, :],
                                     mybir.ActivationFunctionType.Ln)
                rec = recp.tile([1, QB], F32, tag="rec")
                nc.scalar.activation(rec[:, :], lnd[:, :],
                                     mybir.ActivationFunctionType.Exp,
                                     scale=-1.0)
                recb = recp.tile([DH, QB], F32, tag="recb")
                nc.gpsimd.partition_broadcast(recb[:, :], rec[:, :])
                nc.vector.tensor_tensor(
                    out=CT[ho:ho + DH, hc, qb * QB:(qb + 1) * QB],
                    in0=av[0:DH, :], in1=recb[:, :],
                    op=mybir.AluOpType.mult)

            # output projection for this qb's token tiles (all heads done)
            for j in range(4 * qb, 4 * qb + 4):
                ys = yp.tile([128, D], F32)
                for n in range(2):
                    ps = pp_mm.tile([128, 512], F32, tag="mm")
                    for cc in range(2):
                        nc.tensor.matmul(
                            ps[:, :],
                            lhsT=CT[:, cc, j * 128:(j + 1) * 128],
                            rhs=fo_sb[:, cc, n * 512:(n + 1) * 512],
                            start=(cc == 0), stop=(cc == 1))
                    nc.vector.tensor_copy(ys[:, n * 512:(n + 1) * 512], ps[:, :])
                nc.sync.dma_start(out=y[j * 128:(j + 1) * 128, :], in_=ys[:, :])

    nc.compile()
    return nc


_NC_CACHE = {}


def _get_nc():
    key = (XDT, TDT, WDT, MDT)
    if key not in _NC_CACHE:
        _NC_CACHE[key] = build_nc()
    return _NC_CACHE[key]


def make_in_maps(q, k, v, ln_g, ln_b, wq_w, wq_b, wk_w, wk_b, wv_w, wv_b, fo_w, fo_b):
    """Host-side shard prep. Folds ln_g/ln_b into projection weights/biases."""
    xnp = _NPDT[XDT]
    wnp = _NPDT[WDT]
    g64 = ln_g.astype(np.float64)
    b64 = ln_b.astype(np.float64)
    in_maps = []
    for c in range(N_CORES):
        b = c // 4
        sl = slice((c % 4) * HS, (c % 4 + 1) * HS)
        m = {
            "xq": np.ascontiguousarray(q[b]).astype(xnp),
            "xk": np.ascontiguousarray(k[b]).astype(xnp),
            "xv": np.ascontiguousarray(v[b]).astype(xnp),
        }
        for nm, w, bias in (("q", wq_w, wq_b), ("k", wk_w, wk_b), ("v", wv_w, wv_b)):
            ws = w[sl].astype(np.float64)          # [256, 1024]
            wg = ws * g64[None, :]                 # fold gamma
            cb = (ws @ b64 + bias[sl].astype(np.float64)).astype(np.float32)
            m["w" + nm] = np.ascontiguousarray(wg.T).astype(wnp)  # [1024, 256]
            if nm == "v":
                m["bv"] = cb.reshape(1, HS)
            else:
                m["b" + nm] = np.ascontiguousarray(cb.reshape(2, 128).T)  # [128, 2]
        m["fo"] = np.ascontiguousarray(fo_w[:, sl].T).astype(wnp)  # [256, 1024]
        in_maps.append(m)
    return in_maps


def run_on_device(in_maps, trace=False):
    nc = _get_nc()
    return bass_utils.run_bass_kernel_spmd(
        nc, in_maps, core_ids=list(range(N_CORES)), trace=trace)


def assemble(res, fo_b):
    """Gather-reduce the row-parallel partials and add the output bias."""
    fo_b64 = np.asarray(fo_b, np.float64)
    out = np.empty((B, S, D), np.float32)
    for b in range(B):
        acc = np.zeros((S, D), np.float64)
        for c in range(b * 4, b * 4 + 4):
            acc += res.results[c]["y"].astype(np.float64)
        out[b] = (acc + fo_b64[None, :]).astype(np.float32)
    return out


def kernel(q, k, v, ln_g, ln_b, wq_w, wq_b, wk_w, wk_b, wv_w, wv_b, fo_w, fo_b):
    q = np.asarray(q, np.float32)
    k = np.asarray(k, np.float32)
    v = np.asarray(v, np.float32)
    in_maps = make_in_maps(q, k, v, np.asarray(ln_g, np.float32),
                           np.asarray(ln_b, np.float32),
                           np.asarray(wq_w, np.float32), np.asarray(wq_b, np.float32),
                           np.asarray(wk_w, np.float32), np.asarray(wk_b, np.float32),
                           np.asarray(wv_w, np.float32), np.asarray(wv_b, np.float32),
                           np.asarray(fo_w, np.float32), np.asarray(fo_b, np.float32))
    res = run_on_device(in_maps)
    return assemble(res, fo_b)
